# revision 9
# baseline (speedup 1.0000x reference)
"""2-layer LSTM decoder for trn2 — 8-way DATA-parallel over batch (no collectives).

Each core owns 64 batch rows and the FULL weights (~200KB/partition SBUF).
Everything transposed: gate/hidden dim on partitions, batch on the free dim.
Per step: gates = sum_k W_chunk.T @ h_chunk as 128x128x64 matmuls accumulated
in gate-major PSUM banks (one bank per gate, 8 h-chunk slots of 64 batch).
b0 rides a constant ones-row folded into the x-chunk stationary; b1 rides the
per-slot activation bias. Teacher-forcing feedback is a (1,64) DVE copy of the
prediction into the x vector; no cross-core traffic at all.
"""
import numpy as np
import ml_dtypes

import concourse.bass as bass
import concourse.mybir as mybir
import concourse.tile as tile
from concourse import bacc

F32 = mybir.dt.float32
BF16 = mybir.dt.bfloat16
F8 = mybir.dt.float8e4
AF = mybir.ActivationFunctionType
ALU = mybir.AluOpType

B, T_FULL, F, H, GE = 512, 168, 32, 1024, 16
N_CORES = 8
BL = B // N_CORES   # 64 local batch
NKH = 8             # hidden-dim k-chunks (1024/128)
NM = 32             # gate m-chunks (4096/128)
NK0 = 1 + NKH       # L0: x-chunk + 8 h-chunks
NK1 = 2 * NKH       # L1: 8 ih + 8 hh chunks


def prep_host(inputs, T):
    inp = {k: np.asarray(v) for k, v in inputs.items()}
    gv_all = inp["group_emb"][inp["group_ids"].astype(np.int64)]  # (B, GE)
    b0 = (inp["b_ih0"] + inp["b_hh0"]).astype(np.float32)         # (4096,)
    b1 = (inp["b_ih1"] + inp["b_hh1"]).astype(np.float32)
    tf_mask = [int(v) for v in np.asarray(inp["tf_mask"]).reshape(-1)][:T]
    b_proj = float(np.asarray(inp["b_proj"]).reshape(-1)[0])

    # --- stationary weights (shared by all cores) ---
    # w_ext[r, c]: contraction row r, gate col c. x-chunk rows: 0=prev_y,
    # 1:33=known, 33:49=gv, 49=ones(bias), 50:128=0.
    w0_ext = np.zeros((NK0 * 128, 4096), np.float32)
    w0_ext[0:49] = inp["W_ih0"].astype(np.float32).T
    w0_ext[49] = b0
    w0_ext[128:1152] = inp["W_hh0"].astype(np.float32).T
    w1_ext = np.concatenate(
        [inp["W_ih1"].astype(np.float32).T, inp["W_hh1"].astype(np.float32).T],
        axis=0)  # (2048, 4096)

    def pack(w_ext, nk):
        a = w_ext.reshape(nk, 128, NM, 128)          # k p m j
        return np.ascontiguousarray(
            a.transpose(1, 0, 2, 3).reshape(128, nk * NM * 128)
        ).astype(ml_dtypes.bfloat16)

    w0h = pack(w0_ext[128:], NKH)
    w0x = np.ascontiguousarray(
        w0_ext[:128].reshape(128, NM * 128)).astype(ml_dtypes.float8_e4m3)
    w1 = pack(w1_ext, NK1)
    b1_sb = np.ascontiguousarray(b1.reshape(NM, 128).T).astype(np.float32)
    wpT = np.ascontiguousarray(
        inp["W_proj"].astype(np.float32)[0].reshape(NKH, 128).T
    ).astype(ml_dtypes.bfloat16)  # (128, 8)

    shared = dict(w0h=w0h, w0x=w0x, w1=w1, b1=b1_sb, wpT=wpT)

    # --- per-core batch slices ---
    tgt = inp["target_y"].astype(np.float32)[:, :, 0]    # (B, Tfull)
    kn = inp["dec_known"].astype(np.float32)             # (B, Tfull, F)
    per_core = []
    for c in range(N_CORES):
        bs = slice(BL * c, BL * (c + 1))
        kne = np.zeros((T, 1 + F, BL), np.float32)
        kne[0, 0, :] = inp["last_enc_consumption"][bs, 0]
        for t in range(1, T):
            if tf_mask[t - 1]:
                kne[t, 0, :] = tgt[bs, t - 1]
        kne[:, 1:1 + F, :] = kn[bs, :T, :].transpose(1, 2, 0)

        def hT(arr, l):  # (128, NKH*BL): [p, k*BL+b] = arr[l, bs][b, 128k+p]
            return np.ascontiguousarray(
                arr[l, bs].reshape(BL, NKH, 128).transpose(2, 1, 0)
                .reshape(128, NKH * BL))

        d = dict(
            kn=np.ascontiguousarray(kne).astype(ml_dtypes.float8_e4m3),
            gv=np.ascontiguousarray(np.vstack([
                gv_all[bs].T, np.ones((1, BL), np.float32)
            ])).astype(ml_dtypes.float8_e4m3),
            h0i=hT(inp["h0"], 0).astype(ml_dtypes.bfloat16),
            h1i=hT(inp["h0"], 1).astype(ml_dtypes.bfloat16),
            c0i=hT(inp["c0"], 0).astype(np.float32),
            c1i=hT(inp["c0"], 1).astype(np.float32),
        )
        per_core.append(d)
    return shared, per_core, tf_mask, b_proj


def build_module(T, tf_mask, b_proj, rep=1):
    nc = bacc.Bacc(target_bir_lowering=False)

    w0h_d = nc.dram_tensor("w0h", [128, NKH * NM * 128], BF16, kind="ExternalInput")
    w0x_d = nc.dram_tensor("w0x", [128, NM * 128], F8, kind="ExternalInput")
    w1_d = nc.dram_tensor("w1", [128, NK1 * NM * 128], BF16, kind="ExternalInput")
    b1_d = nc.dram_tensor("b1", [128, NM], F32, kind="ExternalInput")
    wpT_d = nc.dram_tensor("wpT", [128, NKH], BF16, kind="ExternalInput")
    kn_d = nc.dram_tensor("kn", [T, 1 + F, BL], F8, kind="ExternalInput")
    gv_d = nc.dram_tensor("gv", [GE + 1, BL], F8, kind="ExternalInput")
    h0i_d = nc.dram_tensor("h0i", [128, NKH * BL], BF16, kind="ExternalInput")
    h1i_d = nc.dram_tensor("h1i", [128, NKH * BL], BF16, kind="ExternalInput")
    c0i_d = nc.dram_tensor("c0i", [128, NKH * BL], F32, kind="ExternalInput")
    c1i_d = nc.dram_tensor("c1i", [128, NKH * BL], F32, kind="ExternalInput")
    out_d = nc.dram_tensor("out", [T, BL], F32, kind="ExternalOutput")

    with tile.TileContext(nc) as tc:
        with tc.tile_pool(name="const", bufs=1) as const, \
             tc.tile_pool(name="hfp", bufs=1) as hfp, \
             tc.tile_pool(name="act", bufs=4) as actp, \
             tc.tile_pool(name="st", bufs=1) as stp, \
             tc.tile_pool(name="tf1", bufs=1, space="PSUM") as tfp, \
             tc.tile_pool(name="sm", bufs=2) as smp, \
             tc.tile_pool(name="gps", bufs=7, space="PSUM") as gpsum:

            w0h_sb = const.tile([128, NKH * NM * 128], BF16)
            nc.sync.dma_start(out=w0h_sb[:], in_=w0h_d[:])
            w0x_sb = const.tile([128, NM * 128], F8)
            nc.sync.dma_start(out=w0x_sb[:], in_=w0x_d[:])
            w1_sb = const.tile([128, NK1 * NM * 128], BF16)
            nc.sync.dma_start(out=w1_sb[:], in_=w1_d[:])
            b1_sb = const.tile([128, NM], F32)
            nc.sync.dma_start(out=b1_sb[:], in_=b1_d[:])
            wpT_sb = const.tile([128, NKH], BF16)
            nc.sync.dma_start(out=wpT_sb[:], in_=wpT_d[:])

            # two x-vector buffers; rows 33:49 = gv, row 49 = 1.0 (bias row)
            xbufs = []
            for i in range(2):
                xb = const.tile([128, BL], F8, name=f"xh0_{i}")
                nc.vector.memset(xb[:], 0.0)
                nc.sync.dma_start(out=xb[33:50, :], in_=gv_d[:])
                xbufs.append(xb)

            def w0h_sl(k, m):
                base = (k * NM + m) * 128
                return w0h_sb[:, base:base + 128]

            def w1_sl(k, m):
                base = (k * NM + m) * 128
                return w1_sb[:, base:base + 128]

            for _rep in range(rep):
                c0_cur = stp.tile([128, NKH, BL], F32, tag="c0", name=f"c0i_{_rep}")
                nc.sync.dma_start(out=c0_cur[:], in_=c0i_d[:])
                c1_cur = stp.tile([128, NKH, BL], F32, tag="c1", name=f"c1i_{_rep}")
                nc.sync.dma_start(out=c1_cur[:], in_=c1i_d[:])
                h0f = hfp.tile([128, NKH, BL], BF16, tag="h0f", name=f"h0i_{_rep}")
                nc.sync.dma_start(out=h0f[:], in_=h0i_d[:])
                h1f = hfp.tile([128, NKH, BL], BF16, tag="h1f", name=f"h1i_{_rep}")
                nc.sync.dma_start(out=h1f[:], in_=h1i_d[:])

                def cell(g, c_cur, ctag, t, b_sb):
                    """g[0..3] = i,f,g,o psum (128,NKH,BL); returns (h, c_new).
                    b_sb None => biases already folded into the psum."""
                    def act(gi, fn, nm):
                        o = actp.tile([128, NKH, BL], BF16, tag="act",
                                      name=f"{nm}_{ctag}_{t}")
                        if b_sb is None:
                            nc.scalar.activation(o[:], g[gi][:], fn)
                        else:
                            for s in range(NKH):
                                nc.scalar.activation(
                                    o[:, s, :], g[gi][:, s, :], fn,
                                    bias=b_sb[:, gi * NKH + s:gi * NKH + s + 1])
                        return o
                    sig_i = act(0, AF.Sigmoid, "si")
                    sig_f = act(1, AF.Sigmoid, "sf")
                    tan_g = act(2, AF.Tanh, "tg")
                    tmpf = tfp.tile([128, NKH, BL], F32, tag="tmpf",
                                    name=f"tf_{ctag}_{t}")
                    nc.vector.tensor_tensor(out=tmpf[:], in0=sig_f[:],
                                            in1=c_cur[:], op=ALU.mult)
                    tmpb = actp.tile([128, NKH, BL], BF16, tag="act",
                                     name=f"tb_{ctag}_{t}")
                    nc.vector.tensor_tensor(out=tmpb[:], in0=sig_i[:],
                                            in1=tan_g[:], op=ALU.mult)
                    sig_o = act(3, AF.Sigmoid, "so")
                    c_new = stp.tile([128, NKH, BL], F32, tag=ctag,
                                     name=f"cn_{ctag}_{t}")
                    nc.vector.tensor_tensor(out=c_new[:], in0=tmpf[:],
                                            in1=tmpb[:], op=ALU.add)
                    tan_c = actp.tile([128, NKH, BL], BF16, tag="act",
                                      name=f"tc_{ctag}_{t}")
                    nc.scalar.activation(tan_c[:], c_new[:], AF.Tanh)
                    hn = hfp.tile([128, NKH, BL], BF16,
                                  tag="h0f" if ctag == "c0" else "h1f",
                                  name=f"hn_{ctag}_{t}")
                    nc.vector.tensor_tensor(out=hn[:], in0=sig_o[:],
                                            in1=tan_c[:], op=ALU.mult)
                    return hn, c_new

                def emit_pred(t, h1_t):
                    pp = gpsum.tile([1, BL], F32, tag="g", name=f"pp_{t}_{_rep}")
                    for k in range(NKH):
                        nc.tensor.matmul(pp[:], wpT_sb[:, k:k + 1], h1_t[:, k, :],
                                         start=(k == 0), stop=(k == NKH - 1))
                    ps = smp.tile([1, BL], F32, tag="pred", name=f"pr_{t}_{_rep}")
                    nc.vector.tensor_scalar_add(ps[:], pp[:], b_proj)
                    nc.sync.dma_start(out=out_d[t:t + 1, :], in_=ps[:])
                    return ps

                for t in range(T):
                    xb = xbufs[t % 2]
                    nc.sync.dma_start(out=xb[0:33, :], in_=kn_d[t])

                    # pred(t-1): runs on PE ahead of the L0 burst; its (1,64)
                    # result lands in xb row 0 well before the x-close below.
                    if t > 0:
                        ps = emit_pred(t - 1, h1f)
                        if not tf_mask[t - 1]:
                            nc.vector.tensor_copy(xb[0:1, :], ps[:])

                    g0 = [gpsum.tile([128, NKH, BL], F32, tag="g",
                                     name=f"g0_{t}_{m}_{_rep}") for m in range(4)]
                    for k in range(1, NK0):
                        for m in range(NM):
                            nc.tensor.matmul(g0[m >> 3][:, m & 7, :],
                                             w0h_sl(k - 1, m), h0f[:, k - 1, :],
                                             start=(k == 1 and (m & 7) == 0),
                                             stop=False)
                    for m in range(NM):
                        nc.tensor.matmul(g0[m >> 3][:, m & 7, :],
                                         w0x_sb[:, m * 128:(m + 1) * 128], xb[:],
                                         start=False, stop=((m & 7) == 7))

                    h0f, c0_cur = cell(g0, c0_cur, "c0", t, None)

                    g1 = [gpsum.tile([128, NKH, BL], F32, tag="g",
                                     name=f"g1_{t}_{m}_{_rep}") for m in range(4)]
                    for k in range(NKH):   # hh chunks first (prev h1f)
                        for m in range(NM):
                            nc.tensor.matmul(g1[m >> 3][:, m & 7, :],
                                             w1_sl(NKH + k, m), h1f[:, k, :],
                                             start=(k == 0 and (m & 7) == 0),
                                             stop=False)
                    for k in range(NKH):   # ih chunks (fresh h0f)
                        for m in range(NM):
                            nc.tensor.matmul(g1[m >> 3][:, m & 7, :],
                                             w1_sl(k, m), h0f[:, k, :],
                                             start=False,
                                             stop=(k == NKH - 1 and (m & 7) == 7))

                    h1f, c1_cur = cell(g1, c1_cur, "c1", t, b1_sb)

                emit_pred(T - 1, h1f)

    nc.finalize()
    return nc


def kernel(**inputs):
    from concourse.bass_utils import run_bass_kernel_spmd
    T = T_FULL
    shared, per_core, tf_mask, b_proj = prep_host(inputs, T)
    nc = build_module(T, tf_mask, b_proj)
    in_maps = []
    for c in range(N_CORES):
        m = dict(shared)
        m.update(per_core[c])
        in_maps.append(m)
    res = run_bass_kernel_spmd(nc, in_maps, list(range(N_CORES)))
    out = np.zeros((B, T, 1), np.float32)
    for c in range(N_CORES):
        out[BL * c:BL * (c + 1), :, 0] = np.asarray(res.results[c]["out"]).T
    return out


# revision 11
# speedup vs baseline: 384.4711x; 384.4711x over previous
"""2-layer LSTM decoder for trn2 — 8-way DATA-parallel over batch, hardware loop.

Each core owns 64 batch rows and the FULL weights (~200KB/partition SBUF).
Transposed layout: gate/hidden dim on partitions, batch on the free dim.
The T-step recurrence is a tc.For_i hardware loop: the ~860-instruction body
is emitted once, so NEFF compile cost is independent of T. Per step:
~800 128x128x64 bf16 matmuls accumulate the two layers' gates into 8 static
PSUM banks (one bank per gate; pred and the f*c temporary reuse idle banks).
b0 rides a constant ones-row folded into the fp8 x-chunk stationary; b1 rides
the per-slot activation bias. Teacher forcing is branchless: a per-step mask
row fm (0/1) gates the prediction feedback into the x-vector, so the loop
body is step-invariant. No cross-core traffic at all.
"""
import numpy as np
import ml_dtypes

import concourse.bass as bass
import concourse.mybir as mybir
import concourse.tile as tile
from concourse import bacc
from concourse.bass import ds

F32 = mybir.dt.float32
BF16 = mybir.dt.bfloat16
F8 = mybir.dt.float8e4
AF = mybir.ActivationFunctionType
ALU = mybir.AluOpType

B, T_FULL, F, H, GE = 512, 168, 32, 1024, 16
N_CORES = 8
BL = B // N_CORES   # 64 local batch
NKH = 8             # hidden-dim k-chunks (1024/128)
NM = 32             # gate m-chunks (4096/128)
NK0 = 1 + NKH       # L0: x-chunk + 8 h-chunks
NK1 = 2 * NKH       # L1: 8 ih + 8 hh chunks


def prep_host(inputs, T):
    inp = {k: np.asarray(v) for k, v in inputs.items()}
    gv_all = inp["group_emb"][inp["group_ids"].astype(np.int64)]  # (B, GE)
    b0 = (inp["b_ih0"] + inp["b_hh0"]).astype(np.float32)         # (4096,)
    b1 = (inp["b_ih1"] + inp["b_hh1"]).astype(np.float32)
    tf_mask = [int(v) for v in np.asarray(inp["tf_mask"]).reshape(-1)][:T]
    b_proj = float(np.asarray(inp["b_proj"]).reshape(-1)[0])

    # --- stationary weights (shared by all cores) ---
    # w_ext[r, c]: contraction row r, gate col c. x-chunk rows: 0=prev_y,
    # 1:33=known, 33:49=gv, 49=ones(bias), 50:128=0.
    w0_ext = np.zeros((NK0 * 128, 4096), np.float32)
    w0_ext[0:49] = inp["W_ih0"].astype(np.float32).T
    w0_ext[49] = b0
    w0_ext[128:1152] = inp["W_hh0"].astype(np.float32).T
    w1_ext = np.concatenate(
        [inp["W_ih1"].astype(np.float32).T, inp["W_hh1"].astype(np.float32).T],
        axis=0)  # (2048, 4096)

    def pack(w_ext, nk):
        a = w_ext.reshape(nk, 128, NM, 128)          # k p m j
        return np.ascontiguousarray(
            a.transpose(1, 0, 2, 3).reshape(128, nk * NM * 128)
        ).astype(ml_dtypes.bfloat16)

    w0h = pack(w0_ext[128:], NKH)
    w0x = np.ascontiguousarray(
        w0_ext[:128].reshape(128, NM * 128)).astype(ml_dtypes.float8_e4m3)
    w1 = pack(w1_ext, NK1)
    b1_sb = np.ascontiguousarray(b1.reshape(NM, 128).T).astype(np.float32)
    wpT = np.ascontiguousarray(
        inp["W_proj"].astype(np.float32)[0].reshape(NKH, 128).T
    ).astype(ml_dtypes.bfloat16)  # (128, 8)

    shared = dict(w0h=w0h, w0x=w0x, w1=w1, b1=b1_sb, wpT=wpT)

    # --- per-core batch slices ---
    tgt = inp["target_y"].astype(np.float32)[:, :, 0]    # (B, Tfull)
    kn = inp["dec_known"].astype(np.float32)             # (B, Tfull, F)
    per_core = []
    for c in range(N_CORES):
        bs = slice(BL * c, BL * (c + 1))
        kne = np.zeros((T + 1, F, BL), np.float32)
        kne[:T] = kn[bs, :T, :].transpose(1, 2, 0)
        # fmk[t, 0, :BL] = teacher-forcing-off mask for step t (0 at t=0)
        # fmk[t, 0, BL:] = known prev_y for step t (last_enc at 0; y[t-1] if tf)
        fmk = np.zeros((T + 1, 1, 2 * BL), np.float32)
        fmk[0, 0, BL:] = inp["last_enc_consumption"][bs, 0]
        for t in range(1, T):
            if tf_mask[t - 1]:
                fmk[t, 0, BL:] = tgt[bs, t - 1]
            else:
                fmk[t, 0, :BL] = 1.0

        def hT(arr, l):  # (128, NKH*BL): [p, k*BL+b] = arr[l, bs][b, 128k+p]
            return np.ascontiguousarray(
                arr[l, bs].reshape(BL, NKH, 128).transpose(2, 1, 0)
                .reshape(128, NKH * BL))

        d = dict(
            kn=np.ascontiguousarray(kne).astype(ml_dtypes.float8_e4m3),
            fmk=np.ascontiguousarray(fmk),
            gv=np.ascontiguousarray(np.vstack([
                gv_all[bs].T, np.ones((1, BL), np.float32)
            ])).astype(ml_dtypes.float8_e4m3),
            h0i=hT(inp["h0"], 0).astype(ml_dtypes.bfloat16),
            h1i=hT(inp["h0"], 1).astype(ml_dtypes.bfloat16),
            c0i=hT(inp["c0"], 0).astype(np.float32),
            c1i=hT(inp["c0"], 1).astype(np.float32),
        )
        per_core.append(d)
    return shared, per_core, tf_mask, b_proj


def build_module(T, tf_mask, b_proj, rep=1):
    nc = bacc.Bacc(target_bir_lowering=False)

    w0h_d = nc.dram_tensor("w0h", [128, NKH * NM * 128], BF16, kind="ExternalInput")
    w0x_d = nc.dram_tensor("w0x", [128, NM * 128], F8, kind="ExternalInput")
    w1_d = nc.dram_tensor("w1", [128, NK1 * NM * 128], BF16, kind="ExternalInput")
    b1_d = nc.dram_tensor("b1", [128, NM], F32, kind="ExternalInput")
    wpT_d = nc.dram_tensor("wpT", [128, NKH], BF16, kind="ExternalInput")
    kn_d = nc.dram_tensor("kn", [T + 1, F, BL], F8, kind="ExternalInput")
    fmk_d = nc.dram_tensor("fmk", [T + 1, 1, 2 * BL], F32, kind="ExternalInput")
    gv_d = nc.dram_tensor("gv", [GE + 1, BL], F8, kind="ExternalInput")
    h0i_d = nc.dram_tensor("h0i", [128, NKH * BL], BF16, kind="ExternalInput")
    h1i_d = nc.dram_tensor("h1i", [128, NKH * BL], BF16, kind="ExternalInput")
    c0i_d = nc.dram_tensor("c0i", [128, NKH * BL], F32, kind="ExternalInput")
    c1i_d = nc.dram_tensor("c1i", [128, NKH * BL], F32, kind="ExternalInput")
    out_d = nc.dram_tensor("out", [T, BL], F32, kind="ExternalOutput")

    with tile.TileContext(nc) as tc:
        with tc.tile_pool(name="const", bufs=1) as const, \
             tc.tile_pool(name="gps", bufs=1, space="PSUM") as gpsum:

            w0h_sb = const.tile([128, NKH * NM * 128], BF16)
            nc.sync.dma_start(out=w0h_sb[:], in_=w0h_d[:])
            w0x_sb = const.tile([128, NM * 128], F8)
            nc.sync.dma_start(out=w0x_sb[:], in_=w0x_d[:])
            w1_sb = const.tile([128, NK1 * NM * 128], BF16)
            nc.sync.dma_start(out=w1_sb[:], in_=w1_d[:])
            b1_sb = const.tile([128, NM], F32)
            nc.sync.dma_start(out=b1_sb[:], in_=b1_d[:])
            wpT_sb = const.tile([128, NKH], BF16)
            nc.sync.dma_start(out=wpT_sb[:], in_=wpT_d[:])

            # x vector: row0 prev_y, 1:33 known, 33:49 gv, 49 ones, 50:128 0
            xb = const.tile([128, BL], F8, name="xh0")
            nc.vector.memset(xb[:], 0.0)
            nc.sync.dma_start(out=xb[33:50, :], in_=gv_d[:])

            # persistent state + scratch
            h0f = const.tile([128, NKH, BL], BF16, name="h0f")
            h1f = const.tile([128, NKH, BL], BF16, name="h1f")
            c0s = const.tile([128, NKH, BL], F32, name="c0s")
            c1s = const.tile([128, NKH, BL], F32, name="c1s")
            a_i = const.tile([128, NKH, BL], BF16, name="a_i")
            a_f = const.tile([128, NKH, BL], BF16, name="a_f")   # also sig_o
            a_g = const.tile([128, NKH, BL], BF16, name="a_g")   # also tan_c
            a_b = const.tile([128, NKH, BL], BF16, name="a_b")   # i*tanh(g)
            fmk_sb = const.tile([1, 2 * BL], F32, name="fmk")
            ps = const.tile([1, BL], F32, name="ps")
            tm = const.tile([1, BL], F32, name="tm")

            # static PSUM banks: one per gate; pred + f*c temps reuse them
            g0 = [gpsum.tile([128, NKH, BL], F32, tag=f"g0_{i}", name=f"g0_{i}")
                  for i in range(4)]
            g1 = [gpsum.tile([128, NKH, BL], F32, tag=f"g1_{i}", name=f"g1_{i}")
                  for i in range(4)]

            def w0h_sl(k, m):
                base = (k * NM + m) * 128
                return w0h_sb[:, base:base + 128]

            def w1_sl(k, m):
                base = (k * NM + m) * 128
                return w1_sb[:, base:base + 128]

            def cell(g, c_cur, ctag, b_sb):
                def act(gi, out, fn):
                    if b_sb is None:
                        nc.scalar.activation(out[:], g[gi][:], fn)
                    else:
                        for s in range(NKH):
                            nc.scalar.activation(
                                out[:, s, :], g[gi][:, s, :], fn,
                                bias=b_sb[:, gi * NKH + s:gi * NKH + s + 1])
                act(0, a_i, AF.Sigmoid)
                act(1, a_f, AF.Sigmoid)
                act(2, a_g, AF.Tanh)
                tmpf = g[1]  # f-gate bank becomes the f*c scratch
                nc.vector.tensor_tensor(out=tmpf[:], in0=a_f[:], in1=c_cur[:],
                                        op=ALU.mult)
                nc.vector.tensor_tensor(out=a_b[:], in0=a_i[:], in1=a_g[:],
                                        op=ALU.mult)
                act(3, a_f, AF.Sigmoid)       # sig_o overwrites a_f
                nc.vector.tensor_tensor(out=c_cur[:], in0=tmpf[:], in1=a_b[:],
                                        op=ALU.add)
                nc.scalar.activation(a_g[:], c_cur[:], AF.Tanh)  # tan_c
                hn = h0f if ctag == "c0" else h1f
                nc.vector.tensor_tensor(out=hn[:], in0=a_f[:], in1=a_g[:],
                                        op=ALU.mult)

            for _rep in range(rep):
                nc.sync.dma_start(out=c0s[:], in_=c0i_d[:])
                nc.sync.dma_start(out=c1s[:], in_=c1i_d[:])
                nc.sync.dma_start(out=h0f[:], in_=h0i_d[:])
                nc.sync.dma_start(out=h1f[:], in_=h1i_d[:])
                # prologue: x vector for step 0
                nc.sync.dma_start(out=xb[1:33, :], in_=kn_d[0])
                nc.sync.dma_start(out=fmk_sb[:], in_=fmk_d[0])
                nc.vector.tensor_copy(xb[0:1, :], fmk_sb[0:1, BL:])

                with tc.For_i(0, T) as t:
                    # L0: h chunks then x-close
                    for k in range(NKH):
                        for m in range(NM):
                            nc.tensor.matmul(g0[m >> 3][:, m & 7, :],
                                             w0h_sl(k, m), h0f[:, k, :],
                                             start=(k == 0 and (m & 7) == 0),
                                             stop=False)
                    for m in range(NM):
                        nc.tensor.matmul(g0[m >> 3][:, m & 7, :],
                                         w0x_sb[:, m * 128:(m + 1) * 128], xb[:],
                                         start=False, stop=((m & 7) == 7))
                    cell(g0, c0s, "c0", None)

                    for k in range(NKH):   # hh chunks (prev h1f)
                        for m in range(NM):
                            nc.tensor.matmul(g1[m >> 3][:, m & 7, :],
                                             w1_sl(NKH + k, m), h1f[:, k, :],
                                             start=(k == 0 and (m & 7) == 0),
                                             stop=False)
                    for k in range(NKH):   # ih chunks (fresh h0f)
                        for m in range(NM):
                            nc.tensor.matmul(g1[m >> 3][:, m & 7, :],
                                             w1_sl(k, m), h0f[:, k, :],
                                             start=False,
                                             stop=(k == NKH - 1 and (m & 7) == 7))
                    cell(g1, c1s, "c1", b1_sb)

                    # pred(t) into a corner of the (now idle) g0 i-gate bank
                    pp = g0[0][0:1, 0, :]
                    for k in range(NKH):
                        nc.tensor.matmul(pp, wpT_sb[:, k:k + 1], h1f[:, k, :],
                                         start=(k == 0), stop=(k == NKH - 1))
                    nc.vector.tensor_scalar_add(ps[:], pp, b_proj)
                    nc.sync.dma_start(out=out_d[ds(t, 1), :], in_=ps[:])

                    # prefetch x vector for step t+1; masked pred feedback
                    nc.sync.dma_start(out=xb[1:33, :], in_=kn_d[ds(t + 1, 1), :, :])
                    nc.sync.dma_start(out=fmk_sb[:], in_=fmk_d[ds(t + 1, 1), :, :])
                    nc.vector.tensor_tensor(out=tm[:], in0=ps[:],
                                            in1=fmk_sb[0:1, 0:BL], op=ALU.mult)
                    nc.vector.tensor_tensor(out=xb[0:1, :], in0=fmk_sb[0:1, BL:],
                                            in1=tm[:], op=ALU.add)

    nc.finalize()
    return nc


def kernel(**inputs):
    from concourse.bass_utils import run_bass_kernel_spmd
    T = T_FULL
    shared, per_core, tf_mask, b_proj = prep_host(inputs, T)
    nc = build_module(T, tf_mask, b_proj)
    in_maps = []
    for c in range(N_CORES):
        m = dict(shared)
        m.update(per_core[c])
        in_maps.append(m)
    res = run_bass_kernel_spmd(nc, in_maps, list(range(N_CORES)))
    out = np.zeros((B, T, 1), np.float32)
    for c in range(N_CORES):
        out[BL * c:BL * (c + 1), :, 0] = np.asarray(res.results[c]["out"]).T
    return out
